# revision 1
# baseline (speedup 1.0000x reference)
"""DenseGIN (3-layer, dense adjacency) Trainium2 Bass kernel, 8-core SPMD.

Problem: x:(4,4096,2,32) f32, adj:(4,4096,4096) f32 binary, mask:(4,4096) bool.
Per layer l: agg = (adj+I) @ xf ; h = relu(agg@Wa+ba); h = BN(h); h = h@Wb+bb;
x = mask*h ; between layers an outer BN is applied at masked nodes.

Sharding: 8 cores = (batch b, node-half). Core (2b+h) owns output nodes
[h*2048,(h+1)*2048) of batch b.

Key design decisions:
- Host pre-transposes A = (adj[b] + I) and slices columns, so the device gets
  adjT[i, k] = A[k, i] in natural row-major layout.  The PE matmul
  out = lhsT.T @ rhs with lhsT = xf node-major tiles [128 nodes, KC chans] and
  rhs = adjT tiles [128 nodes_in, 512 nodes_out] then directly produces
  aggT[chan, node_out] with NO transposes anywhere on device.
- adjT is cast to bf16 on host (exact: entries are 0/1) and kept RESIDENT in
  SBUF (16 MiB) across all three layers -> adjacency is read from HBM once.
- xf is split into hi+lo bf16 parts (pseudo-fp32): agg accumulates
  A@hi + A@lo in fp32 PSUM; quantization error ~2^-17 relative.
- All BN affines are folded on the host: Wb'' = diag(s1_bn)*Wb*diag(s2_outer),
  d = (bb + c1@Wb)*s2 + c2, so the device epilogue per node tile is just
  (psum + D) * mask_column.
- Between layers, node halves are exchanged with a 2-core AllGather of the
  packed [2048, 256] bf16 (hi|lo) activation tensor.
"""

import sys

if "/opt/trn_rl_repo" not in sys.path:  # PYTHONPATH normally provides it
    sys.path.insert(0, "/opt/trn_rl_repo")

import contextlib
import ctypes
import types

import numpy as np
import ml_dtypes

import concourse.bass as bass
import concourse.tile as tile
from concourse import mybir
from concourse.vector_clock import ScopedClock
import concourse.bass_utils as bass_utils
from concourse.bass_utils import run_bass_kernel_spmd

# ---------------------------------------------------------------------------
# Workaround: the walrus build in this container rejects instructions with
# more than one sem wait ("Too many sync wait commands").  Tile's final drain
# attaches one wait per live semaphore; split them across chained SP drains.
_MAX_WAITS_PER_INST = 1


def _patched_drain_and_barrier(self, tick_clock, wait_clock):
    nc = self.nc
    drain_inst = nc.sync.drain()
    wait_clock.add_sem_waits(drain_inst.ins, ScopedClock({None: tick_clock.global_clock}))
    si = drain_inst.ins.sync_info
    waits = list(si.on_wait or [])
    if len(waits) > _MAX_WAITS_PER_INST:
        si.on_wait = waits[:_MAX_WAITS_PER_INST]
        rest = waits[_MAX_WAITS_PER_INST:]
        for i in range(0, len(rest), _MAX_WAITS_PER_INST):
            extra = nc.sync.drain()
            extra.ins.sync_info = mybir.SyncInfo(
                on_wait=rest[i : i + _MAX_WAITS_PER_INST], on_update=[]
            )
    nc.all_engine_barrier()
    assert self.sems is not None
    popped = nc._tile_sem_poison_stack.pop()
    assert popped is self._sem_poison
    nc.clear_and_free_semaphores(list(self.sems.allocated().values()))
    nc.all_engine_barrier()


tile.TileContext._drain_and_barrier = _patched_drain_and_barrier


def _legalize_sync_waits(nc, max_waits=_MAX_WAITS_PER_INST):
    """Split instructions carrying more than ``max_waits`` sem waits.

    Engine sequencers process their instruction stream in order and execute
    sem waits before dispatch, so hoisting excess waits onto NoOps placed
    just before the instruction (same engine) is semantics-preserving.
    """
    n_split = 0
    for fn in nc.m.functions:
        for blk in fn.blocks:
            insts = blk.instructions
            i = 0
            while i < len(insts):
                inst = insts[i]
                si = inst.sync_info
                waits = list(si.on_wait) if si and si.on_wait else []
                if len(waits) > max_waits:
                    extra, keep = waits[:-max_waits], waits[-max_waits:]
                    si.on_wait = keep
                    pos = i
                    for j in range(0, len(extra), max_waits):
                        nop = mybir.InstNoOp(name=f"I-lsw{n_split}-{j}", ins=[], outs=[])
                        nop.engine = inst.engine
                        nop.sync_info = mybir.SyncInfo(
                            on_wait=extra[j : j + max_waits], on_update=[]
                        )
                        insts.insert(pos, nop)
                        pos += 1
                        i += 1
                    n_split += 1
                i += 1
    return n_split


# ---------------------------------------------------------------------------
# NTFF profiling hook (antenv.axon_hooks is absent in this image).  Only used
# when run() is called with trace=True; registering it is harmless otherwise.
def _ntff_profile_via_ctypes(so_path):
    try:
        lib = ctypes.CDLL(so_path)
    except OSError:
        return None
    if not hasattr(lib, "axon_start_nrt_profile"):
        return None
    lib.axon_start_nrt_profile.argtypes = [ctypes.POINTER(ctypes.c_int64), ctypes.c_size_t]
    lib.axon_start_nrt_profile.restype = ctypes.c_int64
    lib.axon_stop_nrt_profile.argtypes = [ctypes.c_char_p]
    lib.axon_stop_nrt_profile.restype = ctypes.c_int64

    @contextlib.contextmanager
    def _hook(output_dir, device_ids):
        import jax

        jax.devices()
        if device_ids:
            ids = (ctypes.c_int64 * len(device_ids))(*device_ids)
            rc = lib.axon_start_nrt_profile(ids, len(device_ids))
        else:
            rc = lib.axon_start_nrt_profile(None, 0)
        if rc != 0:
            raise RuntimeError(f"axon_start_nrt_profile rc={rc}")
        try:
            yield
        finally:
            n = lib.axon_stop_nrt_profile(str(output_dir).encode())
            print(f"ntff profile: {n} file(s) written to {output_dir}", file=sys.stderr)

    return _hook


if "antenv.axon_hooks" not in sys.modules:
    _hooks_mod = types.ModuleType("antenv.axon_hooks")
    _hook_inst = _ntff_profile_via_ctypes("/opt/axon/libaxon_pjrt.so")
    _hooks_mod.get_axon_ntff_profile_hook = lambda: _hook_inst
    sys.modules["antenv.axon_hooks"] = _hooks_mod
bass_utils.upload_artifacts = lambda tmpdir: f"local:{tmpdir}"

# ---------------------------------------------------------------------------
B, N, K, C_IN, H, C_OUT = 4, 4096, 2, 32, 64, 32
BN_EPS = 1e-5
N_CORES = 8
HALF = N // 2          # 2048 output nodes per core
NT = N // 128          # 32 node tiles (contraction side)
KC_IN = [K * C_IN, K * H, K * H]     # flat input channels per layer: 64,128,128
KC_OUT = [K * H, K * H, K * C_OUT]   # flat output channels per layer: 128,128,64
CO = [H, H, C_OUT]                   # per-k output channels: 64,64,32
CI = [C_IN, H, H]                    # per-k input channels: 32,64,64

BF16 = ml_dtypes.bfloat16

_PROGRAM_CACHE = {}


def _build_program(n_layers=3, use_cc=True):
    """Build the SPMD Bass/Tile program (identical on all 8 cores)."""
    nc = bass.Bass("TRN2", target_bir_lowering=False, debug=False, num_devices=N_CORES)
    dt = mybir.dt

    adjT_d = nc.dram_tensor("adjT", [N, HALF], dt.bfloat16, kind="ExternalInput").ap()
    xh0_d = nc.dram_tensor("xh0", [N, KC_IN[0]], dt.bfloat16, kind="ExternalInput").ap()
    xl0_d = nc.dram_tensor("xl0", [N, KC_IN[0]], dt.bfloat16, kind="ExternalInput").ap()
    mask_d = nc.dram_tensor("mask_cols", [128, 16], dt.float32, kind="ExternalInput").ap()
    # Wa/Wb are stored block-diagonally over the K=2 slice structure so each
    # MLP stage is a single full-partition matmul with base_partition 0
    # (partition-offset matmul operands crash at runtime on this stack).
    wa_d = [
        nc.dram_tensor(f"wa{l}", [KC_IN[l], 2 * H], dt.float32, kind="ExternalInput").ap()
        for l in range(3)
    ]
    wb_d = [
        nc.dram_tensor(f"wb{l}", [2 * H, KC_OUT[l]], dt.float32, kind="ExternalInput").ap()
        for l in range(3)
    ]
    ba_d = [
        nc.dram_tensor(f"ba{l}", [128, 1], dt.float32, kind="ExternalInput").ap()
        for l in range(3)
    ]
    dd_d = [
        nc.dram_tensor(f"d{l}", [128, KC_OUT[l]], dt.float32, kind="ExternalInput").ap()
        for l in range(3)
    ]
    out_d = nc.dram_tensor(
        "out", [HALF, KC_OUT[n_layers - 1]], dt.float32, kind="ExternalOutput"
    ).ap()

    with tile.TileContext(nc) as tc:
        with (
            tc.tile_pool(name="const", bufs=1) as cpool,
            tc.tile_pool(name="xf", bufs=2) as xpool,
            tc.tile_pool(name="work", bufs=3) as wpool,
            tc.tile_pool(name="ps_agg", bufs=2, space="PSUM") as ps_agg,
            tc.tile_pool(name="ps_mlp", bufs=2, space="PSUM") as ps_mlp,
            tc.tile_pool(name="dram", bufs=2, space="DRAM") as dpool,
        ):
            # --- HAM warmup: dummy matmuls keep the PE clock at 8/8 while the
            # initial adjacency DMA streams in (operand contents irrelevant) ---
            wu_lhs = cpool.tile([128, 128], dt.bfloat16, tag="wu_lhs")
            wu_rhs = cpool.tile([128, 512], dt.bfloat16, tag="wu_rhs")
            nc.gpsimd.memset(wu_lhs[:], 0.0)
            nc.gpsimd.memset(wu_rhs[:], 0.0)
            wu_ps = ps_mlp.tile([128, 512], dt.float32, tag="h1")
            for _ in range(28):
                nc.tensor.matmul(wu_ps[:], wu_lhs[:], wu_rhs[:], start=True, stop=True)

            # --- layer-0 activations first (small, unblocks first matmuls) ---
            kc0 = KC_IN[0]
            xh_sb = [
                xpool.tile([128, kc0], dt.bfloat16, tag=f"xh{i}", name=f"xh0_{i}")
                for i in range(NT)
            ]
            xl_sb = [
                xpool.tile([128, kc0], dt.bfloat16, tag=f"xl{i}", name=f"xl0_{i}")
                for i in range(NT)
            ]
            for i in range(NT):
                nc.gpsimd.dma_start(xh_sb[i][:], xh0_d[i * 128 : (i + 1) * 128, :])
                nc.gpsimd.dma_start(xl_sb[i][:], xl0_d[i * 128 : (i + 1) * 128, :])

            # --- resident adjacency: 32 x [128, 2048] bf16 = 16 MiB total.
            # Separate tiles so each accumulation matmul only depends on its
            # own slice's DMA (whole-tile deps would serialize layer 0). ---
            adjT_sb = [
                cpool.tile([128, HALF], dt.bfloat16, tag=f"adjT{i}", name=f"adjT_{i}")
                for i in range(NT)
            ]
            for i in range(NT):
                nc.sync.dma_start(adjT_sb[i][:], adjT_d[i * 128 : (i + 1) * 128, :])

            # --- constants ---
            mask_sb = cpool.tile([128, 16], dt.float32, tag="mask")
            nc.gpsimd.dma_start(mask_sb[:], mask_d[:])
            wa_sb, wb_sb, ba_sb, dd_sb = [], [], [], []
            for l in range(3):
                wa = cpool.tile([KC_IN[l], 2 * H], dt.float32, tag=f"wa{l}")
                nc.gpsimd.dma_start(wa[:], wa_d[l][:])
                wa_sb.append(wa)
                wb = cpool.tile([2 * H, KC_OUT[l]], dt.float32, tag=f"wb{l}")
                nc.gpsimd.dma_start(wb[:], wb_d[l][:])
                wb_sb.append(wb)
                ba = cpool.tile([128, 1], dt.float32, tag=f"ba{l}")
                nc.gpsimd.dma_start(ba[:], ba_d[l][:])
                ba_sb.append(ba)
                dd = cpool.tile([128, KC_OUT[l]], dt.float32, tag=f"d{l}")
                nc.gpsimd.dma_start(dd[:], dd_d[l][:])
                dd_sb.append(dd)

            for l in range(n_layers):
                kci, kco = KC_IN[l], KC_OUT[l]
                last = l == n_layers - 1
                if not last:
                    # one AllGather per 512-node chunk, launched as soon as the
                    # chunk's epilogue finishes, so collective latency hides
                    # behind the remaining chunks' compute.  Output rows are
                    # global-node-indexed: [0:512]=pair-rank0's chunk,
                    # [512:1024]=rank1's chunk -> identical addressing on all
                    # cores (the NEFF is shared).
                    ag_in = [
                        dpool.tile([512, 2 * kco], dt.bfloat16, tag=f"ag_in{c}", name=f"ag_in_l{l}_{c}")
                        for c in range(4)
                    ]
                    ag_out = [
                        dpool.tile([1024, 2 * kco], dt.bfloat16, tag=f"ag_out{c}", name=f"ag_out_l{l}_{c}")
                        for c in range(4)
                    ]

                # contraction order: layer 0 streams tiles in DMA order; later
                # layers consume in chunk-arrival order (AG_c completes ~in
                # launch order), both halves of each chunk together.
                if l == 0:
                    i_order = list(range(NT))
                else:
                    i_order = []
                    for c in range(4):
                        i_order += [c * 4 + t for t in range(4)]
                        i_order += [16 + c * 4 + t for t in range(4)]

                if not last:
                    kcn = KC_IN[l + 1]
                    assert kcn == kco
                    xh_next = [None] * NT
                    xl_next = [None] * NT

                for kc in range(4):
                    # ---- aggregation for this 512-node output chunk:
                    # aggT[chan, node] = sum_i xf_i.T @ adjT_i ----
                    agg_ps = ps_agg.tile(
                        [kci, 512], dt.float32, tag="agg", name=f"agg_l{l}_{kc}"
                    )
                    for ii, i in enumerate(i_order):
                        rhs = adjT_sb[i][:, kc * 512 : (kc + 1) * 512]
                        nc.tensor.matmul(
                            agg_ps[:], xh_sb[i][:], rhs, start=(ii == 0), stop=False
                        )
                        nc.tensor.matmul(
                            agg_ps[:], xl_sb[i][:], rhs, start=False, stop=(ii == NT - 1)
                        )
                    agg_sb = wpool.tile([kci, 512], dt.float32, tag="agg_sb")
                    nc.scalar.copy(agg_sb[:], agg_ps[:])

                    # ---- MLP matmul 1 (block-diagonal Wa) + bias + relu ----
                    h1_ps = ps_mlp.tile([128, 512], dt.float32, tag="h1")
                    nc.tensor.matmul(
                        h1_ps[:], wa_sb[l][:], agg_sb[:], start=True, stop=True
                    )
                    h1_sb = wpool.tile([128, 512], dt.float32, tag="h1_sb")
                    nc.scalar.activation(
                        h1_sb[:],
                        h1_ps[:],
                        mybir.ActivationFunctionType.Relu,
                        bias=ba_sb[l][:, 0:1],
                    )

                    # ---- MLP matmul 2 (node-major) + epilogue per 128-node tile ----
                    for t in range(4):
                        nsl = slice(t * 128, (t + 1) * 128)
                        xn_ps = ps_mlp.tile([128, kco], dt.float32, tag="xn")
                        nc.tensor.matmul(
                            xn_ps[:], h1_sb[:, nsl], wb_sb[l][:], start=True, stop=True
                        )
                        # (psum + D) * mask
                        xn_sb = wpool.tile([128, kco], dt.float32, tag="xn_sb")
                        nc.vector.tensor_add(xn_sb[:], xn_ps[:], dd_sb[l][:])
                        xm_sb = wpool.tile([128, kco], dt.float32, tag="xm_sb")
                        mcol = mask_sb[:, kc * 4 + t : kc * 4 + t + 1]
                        nc.scalar.activation(
                            xm_sb[:],
                            xn_sb[:],
                            mybir.ActivationFunctionType.Copy,
                            scale=mcol,
                        )
                        rows = slice((kc * 4 + t) * 128, (kc * 4 + t + 1) * 128)
                        if not last:
                            hi_sb = wpool.tile([128, kco], dt.bfloat16, tag="hi")
                            nc.vector.tensor_copy(hi_sb[:], xm_sb[:])
                            hif_sb = wpool.tile([128, kco], dt.float32, tag="hif")
                            nc.vector.tensor_copy(hif_sb[:], hi_sb[:])
                            lo_sb = wpool.tile([128, kco], dt.bfloat16, tag="lo")
                            nc.vector.tensor_sub(lo_sb[:], xm_sb[:], hif_sb[:])
                            trows = slice(t * 128, (t + 1) * 128)
                            nc.sync.dma_start(ag_in[kc][trows, 0:kco], hi_sb[:])
                            nc.sync.dma_start(ag_in[kc][trows, kco : 2 * kco], lo_sb[:])
                        else:
                            nc.sync.dma_start(out_d[rows, :], xm_sb[:])

                    if not last:
                        if use_cc:
                            nc.gpsimd.collective_compute(
                                "AllGather",
                                mybir.AluOpType.bypass,
                                replica_groups=[[0, 1], [2, 3], [4, 5], [6, 7]],
                                ins=[ag_in[kc].opt()],
                                outs=[ag_out[kc].opt()],
                            )
                        else:
                            nc.sync.dma_start(ag_out[kc][0:512, :], ag_in[kc][:, :])
                        # next layer's lhsT tiles for this chunk (both halves)
                        for t in range(4):
                            srows = slice(t * 128, (t + 1) * 128)
                            prows = slice(512 + t * 128, 512 + (t + 1) * 128)
                            j0, j1 = kc * 4 + t, 16 + kc * 4 + t
                            xh_next[j0] = xpool.tile([128, kcn], dt.bfloat16, tag=f"xh{j0}", name=f"xh_l{l}_{j0}")
                            xl_next[j0] = xpool.tile([128, kcn], dt.bfloat16, tag=f"xl{j0}", name=f"xl_l{l}_{j0}")
                            xh_next[j1] = xpool.tile([128, kcn], dt.bfloat16, tag=f"xh{j1}", name=f"xh_l{l}_{j1}")
                            xl_next[j1] = xpool.tile([128, kcn], dt.bfloat16, tag=f"xl{j1}", name=f"xl_l{l}_{j1}")
                            nc.gpsimd.dma_start(xh_next[j0][:], ag_out[kc][srows, 0:kcn])
                            nc.gpsimd.dma_start(xl_next[j0][:], ag_out[kc][srows, kcn : 2 * kcn])
                            nc.gpsimd.dma_start(xh_next[j1][:], ag_out[kc][prows, 0:kcn])
                            nc.gpsimd.dma_start(xl_next[j1][:], ag_out[kc][prows, kcn : 2 * kcn])

                if not last:
                    xh_sb, xl_sb = xh_next, xl_next

    n_split = _legalize_sync_waits(nc)
    print(f"kernel: legalized {n_split} multi-wait instructions", file=sys.stderr)
    return nc


def get_program():
    if "nc" not in _PROGRAM_CACHE:
        _PROGRAM_CACHE["nc"] = _build_program()
    return _PROGRAM_CACHE["nc"]


def prepare_in_maps(inputs):
    """Host-side prep: fold BN into weights, transpose+slice adjacency, split x."""
    f32 = np.float32
    x = np.asarray(inputs["x"], f32)
    adj = np.asarray(inputs["adj"], f32)
    mask = np.asarray(inputs["mask"]).astype(bool)

    # folded per-layer constants (shared by all cores)
    const = {}
    for l in range(3):
        Wa = np.asarray(inputs[f"Wa{l}"], f32)
        ba = np.asarray(inputs[f"ba{l}"], f32)
        Wb = np.asarray(inputs[f"Wb{l}"], f32)
        bb = np.asarray(inputs[f"bb{l}"], f32)
        s1 = np.asarray(inputs[f"bng{l}"], f32) / np.sqrt(
            np.asarray(inputs[f"bnv{l}"], f32) + BN_EPS
        )
        c1 = np.asarray(inputs[f"bnb{l}"], f32) - np.asarray(inputs[f"bnm{l}"], f32) * s1
        Wb1 = s1[:, None] * Wb
        bb1 = bb + c1 @ Wb
        if l < 2:
            s2 = np.asarray(inputs[f"og{l}"], f32) / np.sqrt(
                np.asarray(inputs[f"ov{l}"], f32) + BN_EPS
            )
            c2 = np.asarray(inputs[f"ob{l}"], f32) - np.asarray(inputs[f"om{l}"], f32) * s2
            Wb2 = (Wb1 * s2[None, :]).astype(f32)
            d = (bb1 * s2 + c2).astype(f32)
        else:
            Wb2 = Wb1.astype(f32)
            d = bb1.astype(f32)
        dtile = np.broadcast_to(
            np.concatenate([d, d])[None, :], (128, 2 * d.shape[0])
        ).copy()
        ci, co = Wa.shape[0], Wb2.shape[1]
        waBD = np.zeros((2 * ci, 2 * H), f32)
        wbBD = np.zeros((2 * H, 2 * co), f32)
        for k in range(2):
            waBD[k * ci : (k + 1) * ci, k * H : (k + 1) * H] = Wa
            wbBD[k * H : (k + 1) * H, k * co : (k + 1) * co] = Wb2
        const[f"wa{l}"] = waBD
        const[f"wb{l}"] = wbBD
        const[f"ba{l}"] = np.concatenate([ba, ba]).reshape(128, 1).astype(f32)
        const[f"d{l}"] = dtile.astype(f32)

    in_maps = []
    for core in range(N_CORES):
        b, half = divmod(core, 2)
        r0 = half * HALF
        # adjT[i, j] = adj[b][r0+j, i] + I  -> natural layout for rhs tiles
        adjT = np.ascontiguousarray(adj[b][r0 : r0 + HALF, :].T)
        adjT[np.arange(HALF) + r0, np.arange(HALF)] += 1.0
        xb = x[b].reshape(N, KC_IN[0])
        xh = xb.astype(BF16)
        xl = (xb - xh.astype(f32)).astype(BF16)
        mhalf = mask[b][r0 : r0 + HALF].astype(f32)
        m = dict(const)
        m["adjT"] = adjT.astype(BF16)
        m["xh0"] = xh
        m["xl0"] = xl
        m["mask_cols"] = np.ascontiguousarray(mhalf.reshape(16, 128).T)
        in_maps.append(m)
    return in_maps


def run(in_maps, trace=False, **kw):
    nc = get_program()
    return run_bass_kernel_spmd(nc, in_maps, list(range(N_CORES)), trace=trace, **kw)


def kernel(**inputs) -> np.ndarray:
    in_maps = prepare_in_maps(inputs)
    res = run(in_maps)
    out = np.zeros((B, N, K, C_OUT), np.float32)
    for core in range(N_CORES):
        b, half = divmod(core, 2)
        r0 = half * HALF
        out[b, r0 : r0 + HALF] = res.results[core]["out"].reshape(HALF, K, C_OUT)
    return out



# revision 3
# speedup vs baseline: 1.4494x; 1.4494x over previous
"""DenseGIN (3-layer, dense adjacency) Trainium2 Bass kernel, 8-core SPMD.

Problem: x:(4,4096,2,32) f32, adj:(4,4096,4096) f32 binary, mask:(4,4096) bool.
Per layer l: agg = (adj+I) @ xf ; h = relu(agg@Wa+ba); h = BN(h); h = h@Wb+bb;
x = mask*h ; between layers an outer BN is applied at masked nodes.

Sharding: 8 cores = (batch b, node-half). Core (2b+h) owns output nodes
[h*2048,(h+1)*2048) of batch b.

Key design decisions (v2):
- adjT = (adj[b]+I).T column slice, stored fp8e4 (exact for 0/1 entries),
  resident in SBUF (8 MiB) across all layers: half the HBM traffic + SBUF of
  the bf16 version.  Matmuls mix fp8 rhs with bf16 lhsT (PE upconverts both
  internally; only fp32 operands must match).
- PE matmul cost is set by the rhs column count only, so the stationary
  operand's free columns are free real estate:
  * Layer 0 (kci=64): x packed as [xh | xl] bf16 in 128 stationary columns;
    ONE matmul per (tile, chunk) computes hi and lo partial aggs in psum
    partitions 0:64 / 64:128.  MLP1 weights are stacked [Wa; Wa] so the
    hi+lo reduction happens inside the MLP matmul for free.  Layer-0
    precision is therefore still ~fp32.
  * Layers 1-2 (kci=128): single bf16 activations (one rounding per layer;
    measured end-to-end rel err ~6e-4 vs 2e-2 budget).
  Total aggregation matmuls: 384 vs 768 in v1.
- Layer 0 runs tile-outer: each arriving adjT tile feeds 4 matmuls (one per
  512-node output chunk, 4 live PSUM banks), so the PE streams at ~full rate
  during the adjacency DMA instead of idling chunk-by-chunk.
- A dummy AllGather is issued at program start so the one-time collective
  bootstrap/rendezvous (~27 us) overlaps the adjacency load.
- Inter-layer exchange: per-chunk 2-core AllGather of [512, kco] bf16
  (single precision, half the v1 payload).
"""

import sys

if "/opt/trn_rl_repo" not in sys.path:  # PYTHONPATH normally provides it
    sys.path.insert(0, "/opt/trn_rl_repo")

import contextlib
import ctypes
import types

import numpy as np
import ml_dtypes

import concourse.bass as bass
import concourse.tile as tile
from concourse import mybir
from concourse.vector_clock import ScopedClock
import concourse.bass_utils as bass_utils
from concourse.bass_utils import run_bass_kernel_spmd

# ---------------------------------------------------------------------------
# Workaround: the walrus build in this container rejects instructions with
# more than one sem wait ("Too many sync wait commands").  Tile's final drain
# attaches one wait per live semaphore; split them across chained SP drains.
_MAX_WAITS_PER_INST = 1


def _patched_drain_and_barrier(self, tick_clock, wait_clock):
    nc = self.nc
    drain_inst = nc.sync.drain()
    wait_clock.add_sem_waits(drain_inst.ins, ScopedClock({None: tick_clock.global_clock}))
    si = drain_inst.ins.sync_info
    waits = list(si.on_wait or [])
    if len(waits) > _MAX_WAITS_PER_INST:
        si.on_wait = waits[:_MAX_WAITS_PER_INST]
        rest = waits[_MAX_WAITS_PER_INST:]
        for i in range(0, len(rest), _MAX_WAITS_PER_INST):
            extra = nc.sync.drain()
            extra.ins.sync_info = mybir.SyncInfo(
                on_wait=rest[i : i + _MAX_WAITS_PER_INST], on_update=[]
            )
    nc.all_engine_barrier()
    assert self.sems is not None
    popped = nc._tile_sem_poison_stack.pop()
    assert popped is self._sem_poison
    nc.clear_and_free_semaphores(list(self.sems.allocated().values()))
    nc.all_engine_barrier()


tile.TileContext._drain_and_barrier = _patched_drain_and_barrier


def _legalize_sync_waits(nc, max_waits=_MAX_WAITS_PER_INST):
    """Split instructions carrying more than ``max_waits`` sem waits.

    Engine sequencers process their instruction stream in order and execute
    sem waits before dispatch, so hoisting excess waits onto NoOps placed
    just before the instruction (same engine) is semantics-preserving.
    """
    n_split = 0
    for fn in nc.m.functions:
        for blk in fn.blocks:
            insts = blk.instructions
            i = 0
            while i < len(insts):
                inst = insts[i]
                si = inst.sync_info
                waits = list(si.on_wait) if si and si.on_wait else []
                if len(waits) > max_waits:
                    extra, keep = waits[:-max_waits], waits[-max_waits:]
                    si.on_wait = keep
                    pos = i
                    for j in range(0, len(extra), max_waits):
                        nop = mybir.InstNoOp(name=f"I-lsw{n_split}-{j}", ins=[], outs=[])
                        nop.engine = inst.engine
                        nop.sync_info = mybir.SyncInfo(
                            on_wait=extra[j : j + max_waits], on_update=[]
                        )
                        insts.insert(pos, nop)
                        pos += 1
                        i += 1
                    n_split += 1
                i += 1
    return n_split


# ---------------------------------------------------------------------------
# NTFF profiling hook (antenv.axon_hooks is absent in this image).  Only used
# when run() is called with trace=True; registering it is harmless otherwise.
def _ntff_profile_via_ctypes(so_path):
    try:
        lib = ctypes.CDLL(so_path)
    except OSError:
        return None
    if not hasattr(lib, "axon_start_nrt_profile"):
        return None
    lib.axon_start_nrt_profile.argtypes = [ctypes.POINTER(ctypes.c_int64), ctypes.c_size_t]
    lib.axon_start_nrt_profile.restype = ctypes.c_int64
    lib.axon_stop_nrt_profile.argtypes = [ctypes.c_char_p]
    lib.axon_stop_nrt_profile.restype = ctypes.c_int64

    @contextlib.contextmanager
    def _hook(output_dir, device_ids):
        import jax

        jax.devices()
        if device_ids:
            ids = (ctypes.c_int64 * len(device_ids))(*device_ids)
            rc = lib.axon_start_nrt_profile(ids, len(device_ids))
        else:
            rc = lib.axon_start_nrt_profile(None, 0)
        if rc != 0:
            raise RuntimeError(f"axon_start_nrt_profile rc={rc}")
        try:
            yield
        finally:
            n = lib.axon_stop_nrt_profile(str(output_dir).encode())
            print(f"ntff profile: {n} file(s) written to {output_dir}", file=sys.stderr)

    return _hook


if "antenv.axon_hooks" not in sys.modules:
    _hooks_mod = types.ModuleType("antenv.axon_hooks")
    _hook_inst = _ntff_profile_via_ctypes("/opt/axon/libaxon_pjrt.so")
    _hooks_mod.get_axon_ntff_profile_hook = lambda: _hook_inst
    sys.modules["antenv.axon_hooks"] = _hooks_mod
bass_utils.upload_artifacts = lambda tmpdir: f"local:{tmpdir}"

# ---------------------------------------------------------------------------
B, N, K, C_IN, H, C_OUT = 4, 4096, 2, 32, 64, 32
BN_EPS = 1e-5
N_CORES = 8
HALF = N // 2          # 2048 output nodes per core
NT = N // 128          # 32 node tiles (contraction side)
KC_OUT = [K * H, K * H, K * C_OUT]   # flat output channels per layer: 128,128,64

BF16 = ml_dtypes.bfloat16
FP8 = ml_dtypes.float8_e4m3  # == mybir float8e4 (TRN FP8_EXP4); 0/1 exact

ADJ_FP8 = True   # adjacency in fp8e4 (half the DMA/SBUF); bf16 fallback

_PROGRAM_CACHE = {}


def _build_program(n_layers=3, use_cc=True):
    """Build the SPMD Bass/Tile program (identical on all 8 cores)."""
    nc = bass.Bass("TRN2", target_bir_lowering=False, debug=False, num_devices=N_CORES)
    dt = mybir.dt
    adj_dt = dt.float8e4 if ADJ_FP8 else dt.bfloat16

    adjT_d = nc.dram_tensor("adjT", [N, HALF], adj_dt, kind="ExternalInput").ap()
    # layer-0 x, packed [xh | xl] bf16 (64+64 cols)
    x0_d = nc.dram_tensor("x0p", [N, 128], dt.bfloat16, kind="ExternalInput").ap()
    mask_d = nc.dram_tensor("mask_cols", [128, 16], dt.float32, kind="ExternalInput").ap()
    # Wa/Wb are stored block-diagonally over the K=2 slice structure so each
    # MLP stage is a single full-partition matmul with base_partition 0
    # (partition-offset matmul operands crash at runtime on this stack).
    # wa0 additionally stacks the block twice (hi/lo reduction in the MM).
    wa_d = [
        nc.dram_tensor(f"wa{l}", [128, 2 * H], dt.float32, kind="ExternalInput").ap()
        for l in range(3)
    ]
    wb_d = [
        nc.dram_tensor(f"wb{l}", [2 * H, KC_OUT[l]], dt.float32, kind="ExternalInput").ap()
        for l in range(3)
    ]
    ba_d = [
        nc.dram_tensor(f"ba{l}", [128, 1], dt.float32, kind="ExternalInput").ap()
        for l in range(3)
    ]
    dd_d = [
        nc.dram_tensor(f"d{l}", [128, KC_OUT[l]], dt.float32, kind="ExternalInput").ap()
        for l in range(3)
    ]
    out_d = nc.dram_tensor(
        "out", [HALF, KC_OUT[n_layers - 1]], dt.float32, kind="ExternalOutput"
    ).ap()

    with tile.TileContext(nc) as tc:
        with (
            tc.tile_pool(name="const", bufs=1) as cpool,
            tc.tile_pool(name="xf", bufs=2) as xpool,
            tc.tile_pool(name="work", bufs=3) as wpool,
            tc.tile_pool(name="ps_agg", bufs=1, space="PSUM") as ps_agg,
            tc.tile_pool(name="ps_mlp", bufs=2, space="PSUM") as ps_mlp,
            tc.tile_pool(name="dram", bufs=2, space="DRAM") as dpool,
        ):
            # --- dummy AllGather: absorbs the one-time collective bootstrap/
            # rendezvous so the first real AG doesn't pay it ---
            if use_cc:
                dummy_in = dpool.tile([2, 16], dt.float32, tag="cc_warm_in", bufs=1)
                dummy_out = dpool.tile([4, 16], dt.float32, tag="cc_warm_out", bufs=1)
                nc.gpsimd.dma_start(dummy_in[:], mask_d[0:2, :])
                nc.gpsimd.collective_compute(
                    "AllGather",
                    mybir.AluOpType.bypass,
                    replica_groups=[[0, 1], [2, 3], [4, 5], [6, 7]],
                    ins=[dummy_in.opt()],
                    outs=[dummy_out.opt()],
                )

            # --- layer-0 activations first (small, unblocks first matmuls) ---
            x0_sb = [
                xpool.tile([128, 128], dt.bfloat16, tag=f"xh{i}", name=f"x0_{i}")
                for i in range(NT)
            ]
            for i in range(NT):
                nc.gpsimd.dma_start(x0_sb[i][:], x0_d[i * 128 : (i + 1) * 128, :])

            # --- resident adjacency: 32 x [128, 2048] tiles.  Separate tiles
            # so each accumulation matmul only depends on its own slice's DMA ---
            adjT_sb = [
                cpool.tile([128, HALF], adj_dt, tag=f"adjT{i}", name=f"adjT_{i}")
                for i in range(NT)
            ]
            for i in range(NT):
                nc.sync.dma_start(adjT_sb[i][:], adjT_d[i * 128 : (i + 1) * 128, :])

            # --- constants ---
            mask_sb = cpool.tile([128, 16], dt.float32, tag="mask")
            nc.gpsimd.dma_start(mask_sb[:], mask_d[:])
            wa_sb, wb_sb, ba_sb, dd_sb = [], [], [], []
            for l in range(3):
                wa = cpool.tile([128, 2 * H], dt.float32, tag=f"wa{l}")
                nc.gpsimd.dma_start(wa[:], wa_d[l][:])
                wa_sb.append(wa)
                wb = cpool.tile([2 * H, KC_OUT[l]], dt.float32, tag=f"wb{l}")
                nc.gpsimd.dma_start(wb[:], wb_d[l][:])
                wb_sb.append(wb)
                ba = cpool.tile([128, 1], dt.float32, tag=f"ba{l}")
                nc.gpsimd.dma_start(ba[:], ba_d[l][:])
                ba_sb.append(ba)
                dd = cpool.tile([128, KC_OUT[l]], dt.float32, tag=f"d{l}")
                nc.gpsimd.dma_start(dd[:], dd_d[l][:])
                dd_sb.append(dd)

            # --- HAM warmup: dummy matmuls prime the PE clock to 8/8 just
            # before the first real (DMA-gated) layer-0 matmuls ---
            wu_lhs = cpool.tile([128, 128], dt.bfloat16, tag="wu_lhs")
            wu_rhs = cpool.tile([128, 512], dt.bfloat16, tag="wu_rhs")
            nc.gpsimd.memset(wu_lhs[:], 0.0)
            nc.gpsimd.memset(wu_rhs[:], 0.0)
            wu_ps = ps_mlp.tile([128, 512], dt.float32, tag="h1")
            for _ in range(8):
                nc.tensor.matmul(wu_ps[:], wu_lhs[:], wu_rhs[:], start=True, stop=True)

            for l in range(n_layers):
                kco = KC_OUT[l]
                last = l == n_layers - 1
                if not last:
                    # one AllGather per 512-node chunk, launched as soon as the
                    # chunk's epilogue finishes, so collective latency hides
                    # behind the remaining chunks' compute.  Output rows are
                    # global-node-indexed: [0:512]=pair-rank0's chunk,
                    # [512:1024]=rank1's chunk -> identical addressing on all
                    # cores (the NEFF is shared).
                    ag_in = [
                        dpool.tile([512, kco], dt.bfloat16, tag=f"ag_in{c}", name=f"ag_in_l{l}_{c}")
                        for c in range(4)
                    ]
                    ag_out = [
                        dpool.tile([1024, kco], dt.bfloat16, tag=f"ag_out{c}", name=f"ag_out_l{l}_{c}")
                        for c in range(4)
                    ]
                    xh_next = [None] * NT

                # ---- aggregation ----
                agg_ps = [
                    ps_agg.tile([128, 512], dt.float32, tag=f"agg{kc}", name=f"agg_l{l}_{kc}")
                    for kc in range(4)
                ]
                if l == 0:
                    # tile-outer: each arriving adjT tile feeds all 4 chunks
                    for i in range(NT):
                        for kc in range(4):
                            nc.tensor.matmul(
                                agg_ps[kc][:],
                                x0_sb[i][:],
                                adjT_sb[i][:, kc * 512 : (kc + 1) * 512],
                                start=(i == 0),
                                stop=(i == NT - 1),
                            )
                else:
                    # chunk-sequential, consuming tiles in AG-arrival order
                    i_order = []
                    for c in range(4):
                        i_order += [c * 4 + t for t in range(4)]
                        i_order += [16 + c * 4 + t for t in range(4)]
                    for kc in range(4):
                        for ii, i in enumerate(i_order):
                            nc.tensor.matmul(
                                agg_ps[kc][:],
                                xh_sb[i][:],
                                adjT_sb[i][:, kc * 512 : (kc + 1) * 512],
                                start=(ii == 0),
                                stop=(ii == NT - 1),
                            )

                # ---- per-chunk epilogue: MLP + mask + (AG | output) ----
                for kc in range(4):
                    agg_sb = wpool.tile([128, 512], dt.float32, tag="agg_sb")
                    nc.scalar.copy(agg_sb[:], agg_ps[kc][:])

                    h1_ps = ps_mlp.tile([128, 512], dt.float32, tag="h1")
                    nc.tensor.matmul(
                        h1_ps[:], wa_sb[l][:], agg_sb[:], start=True, stop=True
                    )
                    h1_sb = wpool.tile([128, 512], dt.float32, tag="h1_sb")
                    nc.scalar.activation(
                        h1_sb[:],
                        h1_ps[:],
                        mybir.ActivationFunctionType.Relu,
                        bias=ba_sb[l][:, 0:1],
                    )

                    for t in range(4):
                        nsl = slice(t * 128, (t + 1) * 128)
                        xn_ps = ps_mlp.tile([128, kco], dt.float32, tag="xn")
                        nc.tensor.matmul(
                            xn_ps[:], h1_sb[:, nsl], wb_sb[l][:], start=True, stop=True
                        )
                        # (psum + D) * mask
                        xn_sb = wpool.tile([128, kco], dt.float32, tag="xn_sb")
                        nc.vector.tensor_add(xn_sb[:], xn_ps[:], dd_sb[l][:])
                        mcol = mask_sb[:, kc * 4 + t : kc * 4 + t + 1]
                        if not last:
                            xm_bf = wpool.tile([128, kco], dt.bfloat16, tag="xm_bf")
                            nc.scalar.activation(
                                xm_bf[:],
                                xn_sb[:],
                                mybir.ActivationFunctionType.Copy,
                                scale=mcol,
                            )
                            trows = slice(t * 128, (t + 1) * 128)
                            nc.sync.dma_start(ag_in[kc][trows, :], xm_bf[:])
                        else:
                            xm_sb = wpool.tile([128, kco], dt.float32, tag="xm_sb")
                            nc.scalar.activation(
                                xm_sb[:],
                                xn_sb[:],
                                mybir.ActivationFunctionType.Copy,
                                scale=mcol,
                            )
                            rows = slice((kc * 4 + t) * 128, (kc * 4 + t + 1) * 128)
                            nc.sync.dma_start(out_d[rows, :], xm_sb[:])

                    if not last:
                        if use_cc:
                            nc.gpsimd.collective_compute(
                                "AllGather",
                                mybir.AluOpType.bypass,
                                replica_groups=[[0, 1], [2, 3], [4, 5], [6, 7]],
                                ins=[ag_in[kc].opt()],
                                outs=[ag_out[kc].opt()],
                            )
                        else:
                            nc.sync.dma_start(ag_out[kc][0:512, :], ag_in[kc][:, :])
                        # next layer's lhsT tiles for this chunk (both halves)
                        for t in range(4):
                            srows = slice(t * 128, (t + 1) * 128)
                            prows = slice(512 + t * 128, 512 + (t + 1) * 128)
                            j0, j1 = kc * 4 + t, 16 + kc * 4 + t
                            xh_next[j0] = xpool.tile([128, kco], dt.bfloat16, tag=f"xh{j0}", name=f"xh_l{l}_{j0}")
                            xh_next[j1] = xpool.tile([128, kco], dt.bfloat16, tag=f"xh{j1}", name=f"xh_l{l}_{j1}")
                            nc.gpsimd.dma_start(xh_next[j0][:], ag_out[kc][srows, :])
                            nc.gpsimd.dma_start(xh_next[j1][:], ag_out[kc][prows, :])

                if not last:
                    xh_sb = xh_next

    n_split = _legalize_sync_waits(nc)
    print(f"kernel: legalized {n_split} multi-wait instructions", file=sys.stderr)
    return nc


def get_program():
    if "nc" not in _PROGRAM_CACHE:
        _PROGRAM_CACHE["nc"] = _build_program()
    return _PROGRAM_CACHE["nc"]


def prepare_in_maps(inputs):
    """Host-side prep: fold BN into weights, transpose+slice adjacency, split x."""
    f32 = np.float32
    x = np.asarray(inputs["x"], f32)
    adj = np.asarray(inputs["adj"], f32)
    mask = np.asarray(inputs["mask"]).astype(bool)

    # folded per-layer constants (shared by all cores)
    const = {}
    for l in range(3):
        Wa = np.asarray(inputs[f"Wa{l}"], f32)
        ba = np.asarray(inputs[f"ba{l}"], f32)
        Wb = np.asarray(inputs[f"Wb{l}"], f32)
        bb = np.asarray(inputs[f"bb{l}"], f32)
        s1 = np.asarray(inputs[f"bng{l}"], f32) / np.sqrt(
            np.asarray(inputs[f"bnv{l}"], f32) + BN_EPS
        )
        c1 = np.asarray(inputs[f"bnb{l}"], f32) - np.asarray(inputs[f"bnm{l}"], f32) * s1
        Wb1 = s1[:, None] * Wb
        bb1 = bb + c1 @ Wb
        if l < 2:
            s2 = np.asarray(inputs[f"og{l}"], f32) / np.sqrt(
                np.asarray(inputs[f"ov{l}"], f32) + BN_EPS
            )
            c2 = np.asarray(inputs[f"ob{l}"], f32) - np.asarray(inputs[f"om{l}"], f32) * s2
            Wb2 = (Wb1 * s2[None, :]).astype(f32)
            d = (bb1 * s2 + c2).astype(f32)
        else:
            Wb2 = Wb1.astype(f32)
            d = bb1.astype(f32)
        dtile = np.broadcast_to(
            np.concatenate([d, d])[None, :], (128, 2 * d.shape[0])
        ).copy()
        ci, co = Wa.shape[0], Wb2.shape[1]
        waBD = np.zeros((2 * ci, 2 * H), f32)
        wbBD = np.zeros((2 * H, 2 * co), f32)
        for k in range(2):
            waBD[k * ci : (k + 1) * ci, k * H : (k + 1) * H] = Wa
            wbBD[k * H : (k + 1) * H, k * co : (k + 1) * co] = Wb2
        if l == 0:
            # layer 0: agg psum rows 0:64 = hi part, 64:128 = lo part; stack
            # the 64-row block-diag Wa twice so the MM reduces hi+lo.
            const["wa0"] = np.vstack([waBD, waBD])
        else:
            const[f"wa{l}"] = waBD
        const[f"wb{l}"] = wbBD
        const[f"ba{l}"] = np.concatenate([ba, ba]).reshape(128, 1).astype(f32)
        const[f"d{l}"] = dtile.astype(f32)

    in_maps = []
    for core in range(N_CORES):
        b, half = divmod(core, 2)
        r0 = half * HALF
        # adjT[i, j] = adj[b][r0+j, i] + I  -> natural layout for rhs tiles
        adjT = np.ascontiguousarray(adj[b][r0 : r0 + HALF, :].T)
        adjT[np.arange(HALF) + r0, np.arange(HALF)] += 1.0
        xb = x[b].reshape(N, K * C_IN)
        xh = xb.astype(BF16)
        xl = (xb - xh.astype(f32)).astype(BF16)
        mhalf = mask[b][r0 : r0 + HALF].astype(f32)
        m = dict(const)
        m["adjT"] = adjT.astype(FP8 if ADJ_FP8 else BF16)
        m["x0p"] = np.ascontiguousarray(np.hstack([xh, xl]))
        m["mask_cols"] = np.ascontiguousarray(mhalf.reshape(16, 128).T)
        in_maps.append(m)
    return in_maps


def run(in_maps, trace=False, **kw):
    nc = get_program()
    return run_bass_kernel_spmd(nc, in_maps, list(range(N_CORES)), trace=trace, **kw)


def kernel(**inputs) -> np.ndarray:
    in_maps = prepare_in_maps(inputs)
    res = run(in_maps)
    out = np.zeros((B, N, K, C_OUT), np.float32)
    for core in range(N_CORES):
        b, half = divmod(core, 2)
        r0 = half * HALF
        out[b, r0 : r0 + HALF] = res.results[core]["out"].reshape(HALF, K, C_OUT)
    return out


# revision 14
# speedup vs baseline: 1.6951x; 1.1695x over previous
"""DenseGIN (3-layer, dense adjacency) Trainium2 Bass kernel, 8-core SPMD.

Problem: x:(4,4096,2,32) f32, adj:(4,4096,4096) f32 binary, mask:(4,4096) bool.
Per layer l: agg = (adj+I) @ xf ; h = relu(agg@Wa+ba); h = BN(h); h = h@Wb+bb;
x = mask*h ; between layers an outer BN is applied at masked nodes.

Sharding: 8 cores = (batch b, node-half). Core (2b+h) owns output nodes
[h*2048,(h+1)*2048) of batch b.

Key design decisions (v3):
- adjT = (adj[b]+I).T column slice, stored fp8e4 (exact for 0/1 entries),
  resident in SBUF (8 MiB): half the HBM traffic + SBUF of bf16.  Matmuls mix
  fp8 rhs with bf16 lhsT (the PE upconverts internally; only fp32 operands
  must match).
- The adjacency is laid out in DRAM *output-chunk-major*: for each 512-node
  output chunk kc, a [128, 32*512] block whose column block i holds adjacency
  rows for contraction tile i.  Chunk 0's slab lands first, so layer 0
  pipelines chunk-by-chunk with the DMA stream (epilogue+AllGather for chunk
  0 launches at ~25% of the load) and the whole load is 8 large DMAs.
- PE matmul cost is set by the rhs column count only, so stationary columns
  are free real estate:
  * Layer 0 (kci=64): x packed as [xh | xl] bf16 in 128 stationary columns;
    ONE matmul per (tile, chunk) computes hi and lo partial aggs; MLP1
    weights are stacked [Wa; Wa] so the hi+lo reduction happens inside the
    MLP matmul.  Layer-0 precision stays ~fp32.
  * Layers 1-2: single bf16 activations (end-to-end rel err ~6e-4 vs 2e-2).
- Activations live in [128, 512] chunk tiles (node-within-tile on partitions,
  4 node-tiles side by side in columns), so inter-layer exchange is ONE
  [128, 512] bf16 AllGather payload per chunk and ONE (+1 peer) SBUF load.
- Engine-queue discipline: sync = bulk input DMA + epilogue stores;
  gpsimd = consts + collective triggers ONLY (so AllGathers pipeline);
  vector = AG-dependent SBUF loads (the only engine that may block on AGs).
- A dummy AllGather issued first absorbs the one-time CC bootstrap (~13 us).
"""

import sys

if "/opt/trn_rl_repo" not in sys.path:  # PYTHONPATH normally provides it
    sys.path.insert(0, "/opt/trn_rl_repo")

import contextlib
import ctypes
import types

import numpy as np
import ml_dtypes

import concourse.bass as bass
import concourse.tile as tile
from concourse import mybir
from concourse.vector_clock import ScopedClock
import concourse.bass_utils as bass_utils
from concourse.bass_utils import run_bass_kernel_spmd

# ---------------------------------------------------------------------------
# Workaround: the walrus build in this container rejects instructions with
# more than one sem wait ("Too many sync wait commands").  Tile's final drain
# attaches one wait per live semaphore; split them across chained SP drains.
_MAX_WAITS_PER_INST = 1


def _patched_drain_and_barrier(self, tick_clock, wait_clock):
    nc = self.nc
    drain_inst = nc.sync.drain()
    wait_clock.add_sem_waits(drain_inst.ins, ScopedClock({None: tick_clock.global_clock}))
    si = drain_inst.ins.sync_info
    waits = list(si.on_wait or [])
    if len(waits) > _MAX_WAITS_PER_INST:
        si.on_wait = waits[:_MAX_WAITS_PER_INST]
        rest = waits[_MAX_WAITS_PER_INST:]
        for i in range(0, len(rest), _MAX_WAITS_PER_INST):
            extra = nc.sync.drain()
            extra.ins.sync_info = mybir.SyncInfo(
                on_wait=rest[i : i + _MAX_WAITS_PER_INST], on_update=[]
            )
    nc.all_engine_barrier()
    assert self.sems is not None
    popped = nc._tile_sem_poison_stack.pop()
    assert popped is self._sem_poison
    nc.clear_and_free_semaphores(list(self.sems.allocated().values()))
    nc.all_engine_barrier()


tile.TileContext._drain_and_barrier = _patched_drain_and_barrier


def _legalize_sync_waits(nc, max_waits=_MAX_WAITS_PER_INST):
    """Split instructions carrying more than ``max_waits`` sem waits.

    Engine sequencers process their instruction stream in order and execute
    sem waits before dispatch, so hoisting excess waits onto NoOps placed
    just before the instruction (same engine) is semantics-preserving.
    """
    n_split = 0
    for fn in nc.m.functions:
        for blk in fn.blocks:
            insts = blk.instructions
            i = 0
            while i < len(insts):
                inst = insts[i]
                si = inst.sync_info
                waits = list(si.on_wait) if si and si.on_wait else []
                if len(waits) > max_waits:
                    extra, keep = waits[:-max_waits], waits[-max_waits:]
                    si.on_wait = keep
                    pos = i
                    for j in range(0, len(extra), max_waits):
                        nop = mybir.InstNoOp(name=f"I-lsw{n_split}-{j}", ins=[], outs=[])
                        nop.engine = inst.engine
                        nop.sync_info = mybir.SyncInfo(
                            on_wait=extra[j : j + max_waits], on_update=[]
                        )
                        insts.insert(pos, nop)
                        pos += 1
                        i += 1
                    n_split += 1
                i += 1
    return n_split


# ---------------------------------------------------------------------------
# NTFF profiling hook (antenv.axon_hooks is absent in this image).  Only used
# when run() is called with trace=True; registering it is harmless otherwise.
def _ntff_profile_via_ctypes(so_path):
    try:
        lib = ctypes.CDLL(so_path)
    except OSError:
        return None
    if not hasattr(lib, "axon_start_nrt_profile"):
        return None
    lib.axon_start_nrt_profile.argtypes = [ctypes.POINTER(ctypes.c_int64), ctypes.c_size_t]
    lib.axon_start_nrt_profile.restype = ctypes.c_int64
    lib.axon_stop_nrt_profile.argtypes = [ctypes.c_char_p]
    lib.axon_stop_nrt_profile.restype = ctypes.c_int64

    @contextlib.contextmanager
    def _hook(output_dir, device_ids):
        import jax

        jax.devices()
        if device_ids:
            ids = (ctypes.c_int64 * len(device_ids))(*device_ids)
            rc = lib.axon_start_nrt_profile(ids, len(device_ids))
        else:
            rc = lib.axon_start_nrt_profile(None, 0)
        if rc != 0:
            raise RuntimeError(f"axon_start_nrt_profile rc={rc}")
        try:
            yield
        finally:
            n = lib.axon_stop_nrt_profile(str(output_dir).encode())
            print(f"ntff profile: {n} file(s) written to {output_dir}", file=sys.stderr)

    return _hook


if "antenv.axon_hooks" not in sys.modules:
    _hooks_mod = types.ModuleType("antenv.axon_hooks")
    _hook_inst = _ntff_profile_via_ctypes("/opt/axon/libaxon_pjrt.so")
    _hooks_mod.get_axon_ntff_profile_hook = lambda: _hook_inst
    sys.modules["antenv.axon_hooks"] = _hooks_mod
bass_utils.upload_artifacts = lambda tmpdir: f"local:{tmpdir}"

# ---------------------------------------------------------------------------
B, N, K, C_IN, H, C_OUT = 4, 4096, 2, 32, 64, 32
BN_EPS = 1e-5
N_CORES = 8
HALF = N // 2          # 2048 output nodes per core
NT = N // 128          # 32 node tiles (contraction side)
KC_OUT = [K * H, K * H, K * C_OUT]   # flat output channels per layer: 128,128,64

BF16 = ml_dtypes.bfloat16
FP8 = ml_dtypes.float8_e4m3  # == mybir float8e4 (TRN FP8_EXP4); 0/1 exact

ADJ_FP8 = True   # adjacency in fp8e4 (half the DMA/SBUF); bf16 fallback

_PROGRAM_CACHE = {}


def _build_program(n_layers=3, use_cc=True):
    """Build the SPMD Bass/Tile program (identical on all 8 cores)."""
    nc = bass.Bass("TRN2", target_bir_lowering=False, debug=False, num_devices=N_CORES)
    dt = mybir.dt
    adj_dt = dt.float8e4 if ADJ_FP8 else dt.bfloat16

    # chunk-major adjacency: row kc*128+p, col i*512+j = (adj+I)[node r0+kc*512+j,
    # node i*128+p].  Chunk kc's slab is one contiguous [128, 16384] block.
    adjc_d = nc.dram_tensor("adjc", [4 * 128, NT * 512], adj_dt, kind="ExternalInput").ap()
    # layer-0 x, packed [xh | xl] bf16, block-permuted: col i*128+c = tile i's
    # packed channel c, partition p = node i*128+p.
    x0_d = nc.dram_tensor("x0p", [128, NT * 128], dt.bfloat16, kind="ExternalInput").ap()
    mask_d = nc.dram_tensor("mask_cols", [128, 16], dt.float32, kind="ExternalInput").ap()
    # Wa/Wb are stored block-diagonally over the K=2 slice structure so each
    # MLP stage is a single full-partition matmul with base_partition 0
    # (partition-offset matmul operands crash at runtime on this stack).
    # wa0 additionally stacks the block twice (hi/lo reduction in the MM).
    wa_d = [
        nc.dram_tensor(f"wa{l}", [128, 2 * H], dt.float32, kind="ExternalInput").ap()
        for l in range(3)
    ]
    wb_d = [
        nc.dram_tensor(f"wb{l}", [2 * H, KC_OUT[l]], dt.float32, kind="ExternalInput").ap()
        for l in range(3)
    ]
    ba_d = [
        nc.dram_tensor(f"ba{l}", [128, 1], dt.float32, kind="ExternalInput").ap()
        for l in range(3)
    ]
    dd_d = [
        nc.dram_tensor(f"d{l}", [128, KC_OUT[l]], dt.float32, kind="ExternalInput").ap()
        for l in range(3)
    ]
    out_d = nc.dram_tensor(
        "out", [HALF, KC_OUT[n_layers - 1]], dt.float32, kind="ExternalOutput"
    ).ap()

    with tile.TileContext(nc) as tc:
        with (
            tc.tile_pool(name="const", bufs=1) as cpool,
            tc.tile_pool(name="xf", bufs=2) as xpool,
            tc.tile_pool(name="work", bufs=3) as wpool,
            tc.tile_pool(name="ps_agg", bufs=1, space="PSUM") as ps_agg,
            tc.tile_pool(name="ps_mlp", bufs=2, space="PSUM") as ps_mlp,
            tc.tile_pool(name="dram", bufs=2, space="DRAM") as dpool,
        ):
            # --- HAM warmup: dummy matmuls prime the PE clock to 8/8 just
            # before the first real layer-0 matmuls ---
            wu_lhs = cpool.tile([128, 128], dt.bfloat16, tag="wu_lhs")
            wu_rhs = cpool.tile([128, 512], dt.bfloat16, tag="wu_rhs")
            nc.gpsimd.memset(wu_lhs[:], 0.0)
            nc.gpsimd.memset(wu_rhs[:], 0.0)
            wu_ps = ps_mlp.tile([128, 512], dt.float32, tag="h1")
            for _ in range(8):
                nc.tensor.matmul(wu_ps[:], wu_lhs[:], wu_rhs[:], start=True, stop=True)

            # --- dummy AllGather first: absorbs the one-time collective
            # bootstrap/rendezvous so the first real AG doesn't pay it ---
            if use_cc:
                dummy_in = dpool.tile([2, 16], dt.float32, tag="cc_warm_in", bufs=1)
                dummy_out = dpool.tile([4, 16], dt.float32, tag="cc_warm_out", bufs=1)
                nc.gpsimd.dma_start(dummy_in[:], mask_d[0:2, :])
                nc.gpsimd.collective_compute(
                    "AllGather",
                    mybir.AluOpType.bypass,
                    replica_groups=[[0, 1], [2, 3], [4, 5], [6, 7]],
                    ins=[dummy_in.opt()],
                    outs=[dummy_out.opt()],
                )

            # --- bulk inputs on the sync queue: x0, then adjacency slabs in
            # consumption order (chunk 0 first, split in halves for earlier
            # first-matmul) ---
            x0_sb = cpool.tile([128, NT * 128], dt.bfloat16, tag="x0")
            nc.sync.dma_start(x0_sb[:], x0_d[:])
            adjc_sb = [
                [
                    cpool.tile([128, 16 * 512], adj_dt, tag=f"adjc{kc}h{h}", name=f"adjc_{kc}_{h}")
                    for h in range(2)
                ]
                for kc in range(4)
            ]
            for kc in range(4):
                for h in range(2):
                    nc.sync.dma_start(
                        adjc_sb[kc][h][:],
                        adjc_d[kc * 128 : (kc + 1) * 128, h * 8192 : (h + 1) * 8192],
                    )

            def adj_rhs(kc, i):
                """rhs [128, 512] for contraction tile i of output chunk kc."""
                h, r = divmod(i, 16)
                return adjc_sb[kc][h][:, r * 512 : (r + 1) * 512]

            # --- constants (gpsimd queue) ---
            mask_sb = cpool.tile([128, 16], dt.float32, tag="mask")
            nc.gpsimd.dma_start(mask_sb[:], mask_d[:])
            wa_sb, wb_sb, ba_sb, dd_sb = [], [], [], []
            for l in range(3):
                wa = cpool.tile([128, 2 * H], dt.float32, tag=f"wa{l}")
                nc.gpsimd.dma_start(wa[:], wa_d[l][:])
                wa_sb.append(wa)
                wb = cpool.tile([2 * H, KC_OUT[l]], dt.float32, tag=f"wb{l}")
                nc.gpsimd.dma_start(wb[:], wb_d[l][:])
                wb_sb.append(wb)
                ba = cpool.tile([128, 1], dt.float32, tag=f"ba{l}")
                nc.gpsimd.dma_start(ba[:], ba_d[l][:])
                ba_sb.append(ba)
                dd = cpool.tile([128, KC_OUT[l]], dt.float32, tag=f"d{l}")
                nc.gpsimd.dma_start(dd[:], dd_d[l][:])
                dd_sb.append(dd)

            # xh[l][kc][r]: [128, 512] bf16 chunk activations feeding layer l
            xh = {l: [[None, None] for _ in range(4)] for l in range(1, n_layers)}
            ag_io = {}  # l -> (ag_in, ag_out) for the boundary after layer l

            # layers >=1 consume the previous boundary's AllGathers in their
            # completion order: chunks 0,1 finish with one-stage skew, then
            # the rushed 3, then 2 (see the stage loop below).
            C_ORDER = [0, 1, 3, 2]

            def emit_agg(l, kc):
                """32 accumulating matmuls for output chunk kc of layer l."""
                agg_ps = ps_agg.tile(
                    [128, 512], dt.float32, tag=f"agg{kc}", name=f"agg_l{l}_{kc}"
                )
                if l == 0:
                    seq = [
                        (i, x0_sb[:, i * 128 : (i + 1) * 128]) for i in range(NT)
                    ]
                else:
                    seq = []
                    for c in C_ORDER:
                        for r in range(2):
                            for t in range(4):
                                i = r * 16 + c * 4 + t
                                seq.append(
                                    (i, xh[l][c][r][:, t * 128 : (t + 1) * 128])
                                )
                for ii, (i, lhsT) in enumerate(seq):
                    nc.tensor.matmul(
                        agg_ps[:],
                        lhsT,
                        adj_rhs(kc, i),
                        start=(ii == 0),
                        stop=(ii == NT - 1),
                    )
                return agg_ps

            def emit_agg_copy(agg_ps):
                # psum -> sbuf drain; emitted AFTER the pending chunk's mlp1
                # so the scalar queue runs [relu(s-1), copy(s)]
                agg_sb = wpool.tile([128, 512], dt.float32, tag="agg_sb")
                nc.scalar.copy(agg_sb[:], agg_ps[:])
                return agg_sb

            def emit_mlp1(l, kc, agg_sb):
                h1_ps = ps_mlp.tile([128, 512], dt.float32, tag="h1")
                nc.tensor.matmul(h1_ps[:], wa_sb[l][:], agg_sb[:], start=True, stop=True)
                h1_sb = wpool.tile([128, 512], dt.float32, tag="h1_sb")
                nc.scalar.activation(
                    h1_sb[:],
                    h1_ps[:],
                    mybir.ActivationFunctionType.Relu,
                    bias=ba_sb[l][:, 0:1],
                )
                return h1_sb

            def emit_mlp2(l, kc, h1_sb):
                kco = KC_OUT[l]
                last = l == n_layers - 1
                if not last:
                    ag_in, ag_out = ag_io[l]
                    xst = wpool.tile([128, 512], dt.bfloat16, tag="xst")
                for t in range(4):
                    nsl = slice(t * 128, (t + 1) * 128)
                    xn_ps = ps_mlp.tile([128, kco], dt.float32, tag="xn", bufs=2)
                    nc.tensor.matmul(
                        xn_ps[:], h1_sb[:, nsl], wb_sb[l][:], start=True, stop=True
                    )
                    # (psum + D) * mask
                    xn_sb = wpool.tile([128, kco], dt.float32, tag="xn_sb")
                    nc.vector.tensor_add(xn_sb[:], xn_ps[:], dd_sb[l][:])
                    mcol = mask_sb[:, kc * 4 + t : kc * 4 + t + 1]
                    if not last:
                        nc.scalar.activation(
                            xst[:, t * 128 : t * 128 + kco],
                            xn_sb[:],
                            mybir.ActivationFunctionType.Copy,
                            scale=mcol,
                        )
                    else:
                        xm_sb = wpool.tile([128, kco], dt.float32, tag="xm_sb")
                        nc.scalar.activation(
                            xm_sb[:],
                            xn_sb[:],
                            mybir.ActivationFunctionType.Copy,
                            scale=mcol,
                        )
                        rows = slice((kc * 4 + t) * 128, (kc * 4 + t + 1) * 128)
                        nc.sync.dma_start(out_d[rows, :], xm_sb[:])

                if not last:
                    nc.sync.dma_start(ag_in[kc][:], xst[:])
                    if use_cc:
                        nc.gpsimd.collective_compute(
                            "AllGather",
                            mybir.AluOpType.bypass,
                            replica_groups=[[0, 1], [2, 3], [4, 5], [6, 7]],
                            ins=[ag_in[kc].opt()],
                            outs=[ag_out[kc].opt()],
                        )
                    else:
                        nc.sync.dma_start(ag_out[kc][0:128, :], ag_in[kc][:, :])
                    # next layer's chunk tiles (both pair halves); sync queue
                    # (never gates collective triggers, and its own later work
                    # is not AG-critical)
                    for r in range(2):
                        xh[l + 1][kc][r] = xpool.tile(
                            [128, 512], dt.bfloat16, tag=f"xh{kc}_{r}", name=f"xh_l{l}_{kc}_{r}"
                        )
                        nc.sync.dma_start(
                            xh[l + 1][kc][r][:], ag_out[kc][r * 128 : (r + 1) * 128, :]
                        )

            for l in range(n_layers - 1):
                ag_io[l] = (
                    [
                        dpool.tile([128, 512], dt.bfloat16, tag=f"ag_in{c}", name=f"ag_in_l{l}_{c}")
                        for c in range(4)
                    ],
                    [
                        dpool.tile([256, 512], dt.bfloat16, tag=f"ag_out{c}", name=f"ag_out_l{l}_{c}")
                        for c in range(4)
                    ],
                )

            # Software-pipelined stage loop.  Normal stages run one-chunk
            # skewed: PE order [32 aggs(s), mlp1(s-1), mlp2(s-1)] so mlp1's
            # scalar-side dependency (the agg copy) completed during the agg
            # burst; mlp2 pays a short relu-latency stall.  The LAST chunk before a layer
            # boundary is "rushed" (epilogue emitted immediately, ~1 us PE
            # stall) because the next layer's first chunk needs its
            # AllGather; the pending chunk's epilogue follows it.
            def emit_epi(l, kc, agg_sb):
                h1_sb = emit_mlp1(l, kc, agg_sb)
                emit_mlp2(l, kc, h1_sb)

            pend = None  # (l, kc, agg_sb)
            for l in range(n_layers):
                rush_layer = l < n_layers - 1
                for kc in range(4):
                    agg_ps = emit_agg(l, kc)
                    h1_pend = None
                    if pend is not None:
                        h1_pend = emit_mlp1(pend[0], pend[1], pend[2])
                    agg_sb = emit_agg_copy(agg_ps)
                    if rush_layer and kc == 3:
                        # rush: own epilogue first (unblocks the boundary AG),
                        # then the pending chunk's mlp2
                        emit_epi(l, kc, agg_sb)
                        if pend is not None:
                            emit_mlp2(pend[0], pend[1], h1_pend)
                        pend = None
                    else:
                        if pend is not None:
                            emit_mlp2(pend[0], pend[1], h1_pend)
                        pend = (l, kc, agg_sb)
            # drain the pipeline (last layer's final chunk)
            if pend is not None:
                emit_epi(pend[0], pend[1], pend[2])

    n_split = _legalize_sync_waits(nc)
    print(f"kernel: legalized {n_split} multi-wait instructions", file=sys.stderr)
    return nc


def get_program():
    if "nc" not in _PROGRAM_CACHE:
        _PROGRAM_CACHE["nc"] = _build_program()
    return _PROGRAM_CACHE["nc"]


def prepare_in_maps(inputs):
    """Host-side prep: fold BN into weights, transpose+slice adjacency, split x."""
    f32 = np.float32
    x = np.asarray(inputs["x"], f32)
    adj = np.asarray(inputs["adj"], f32)
    mask = np.asarray(inputs["mask"]).astype(bool)

    # folded per-layer constants (shared by all cores)
    const = {}
    for l in range(3):
        Wa = np.asarray(inputs[f"Wa{l}"], f32)
        ba = np.asarray(inputs[f"ba{l}"], f32)
        Wb = np.asarray(inputs[f"Wb{l}"], f32)
        bb = np.asarray(inputs[f"bb{l}"], f32)
        s1 = np.asarray(inputs[f"bng{l}"], f32) / np.sqrt(
            np.asarray(inputs[f"bnv{l}"], f32) + BN_EPS
        )
        c1 = np.asarray(inputs[f"bnb{l}"], f32) - np.asarray(inputs[f"bnm{l}"], f32) * s1
        Wb1 = s1[:, None] * Wb
        bb1 = bb + c1 @ Wb
        if l < 2:
            s2 = np.asarray(inputs[f"og{l}"], f32) / np.sqrt(
                np.asarray(inputs[f"ov{l}"], f32) + BN_EPS
            )
            c2 = np.asarray(inputs[f"ob{l}"], f32) - np.asarray(inputs[f"om{l}"], f32) * s2
            Wb2 = (Wb1 * s2[None, :]).astype(f32)
            d = (bb1 * s2 + c2).astype(f32)
        else:
            Wb2 = Wb1.astype(f32)
            d = bb1.astype(f32)
        dtile = np.broadcast_to(
            np.concatenate([d, d])[None, :], (128, 2 * d.shape[0])
        ).copy()
        ci, co = Wa.shape[0], Wb2.shape[1]
        waBD = np.zeros((2 * ci, 2 * H), f32)
        wbBD = np.zeros((2 * H, 2 * co), f32)
        for k in range(2):
            waBD[k * ci : (k + 1) * ci, k * H : (k + 1) * H] = Wa
            wbBD[k * H : (k + 1) * H, k * co : (k + 1) * co] = Wb2
        if l == 0:
            # layer 0: agg psum rows 0:64 = hi part, 64:128 = lo part; stack
            # the 64-row block-diag Wa twice so the MM reduces hi+lo.
            const["wa0"] = np.vstack([waBD, waBD])
        else:
            const[f"wa{l}"] = waBD
        const[f"wb{l}"] = wbBD
        const[f"ba{l}"] = np.concatenate([ba, ba]).reshape(128, 1).astype(f32)
        const[f"d{l}"] = dtile.astype(f32)

    in_maps = []
    for core in range(N_CORES):
        b, half = divmod(core, 2)
        r0 = half * HALF
        # adjT[i, j] = adj[b][r0+j, i] + I
        adjT = np.ascontiguousarray(adj[b][r0 : r0 + HALF, :].T)
        adjT[np.arange(HALF) + r0, np.arange(HALF)] += 1.0
        adjT = adjT.astype(FP8 if ADJ_FP8 else BF16)
        # chunk-major slabs: adjc[kc*128+p, i*512+j] = adjT[i*128+p, kc*512+j]
        adjc = np.ascontiguousarray(
            adjT.reshape(NT, 128, 4, 512).transpose(2, 1, 0, 3).reshape(4 * 128, NT * 512)
        )
        xb = x[b].reshape(N, K * C_IN)
        xh = xb.astype(BF16)
        xl = (xb - xh.astype(f32)).astype(BF16)
        x0p = np.hstack([xh, xl])  # [4096, 128]
        # block-permuted: x0pp[p, i*128+c] = x0p[i*128+p, c]
        x0pp = np.ascontiguousarray(
            x0p.reshape(NT, 128, 128).transpose(1, 0, 2).reshape(128, NT * 128)
        )
        mhalf = mask[b][r0 : r0 + HALF].astype(f32)
        m = dict(const)
        m["adjc"] = adjc
        m["x0p"] = x0pp
        m["mask_cols"] = np.ascontiguousarray(mhalf.reshape(16, 128).T)
        in_maps.append(m)
    return in_maps


def run(in_maps, trace=False, **kw):
    nc = get_program()
    return run_bass_kernel_spmd(nc, in_maps, list(range(N_CORES)), trace=trace, **kw)


def kernel(**inputs) -> np.ndarray:
    in_maps = prepare_in_maps(inputs)
    res = run(in_maps)
    out = np.zeros((B, N, K, C_OUT), np.float32)
    for core in range(N_CORES):
        b, half = divmod(core, 2)
        r0 = half * HALF
        out[b, r0 : r0 + HALF] = res.results[core]["out"].reshape(HALF, K, C_OUT)
    return out


# revision 16
# speedup vs baseline: 1.7045x; 1.0055x over previous
"""DenseGIN (3-layer, dense adjacency) Trainium2 Bass kernel, 8-core SPMD.

Problem: x:(4,4096,2,32) f32, adj:(4,4096,4096) f32 binary, mask:(4,4096) bool.
Per layer l: agg = (adj+I) @ xf ; h = relu(agg@Wa+ba); h = BN(h); h = h@Wb+bb;
x = mask*h ; between layers an outer BN is applied at masked nodes.

Sharding: 8 cores = (batch b, node-half). Core (2b+h) owns output nodes
[h*2048,(h+1)*2048) of batch b.

Key design decisions (v3):
- adjT = (adj[b]+I).T column slice, stored fp8e4 (exact for 0/1 entries),
  resident in SBUF (8 MiB): half the HBM traffic + SBUF of bf16.  Matmuls mix
  fp8 rhs with bf16 lhsT (the PE upconverts internally; only fp32 operands
  must match).
- The adjacency is laid out in DRAM *output-chunk-major*: for each 512-node
  output chunk kc, a [128, 32*512] block whose column block i holds adjacency
  rows for contraction tile i.  Chunk 0's slab lands first, so layer 0
  pipelines chunk-by-chunk with the DMA stream (epilogue+AllGather for chunk
  0 launches at ~25% of the load) and the whole load is 8 large DMAs.
- PE matmul cost is set by the rhs column count only, so stationary columns
  are free real estate:
  * Layer 0 (kci=64): x packed as [xh | xl] bf16 in 128 stationary columns;
    ONE matmul per (tile, chunk) computes hi and lo partial aggs; MLP1
    weights are stacked [Wa; Wa] so the hi+lo reduction happens inside the
    MLP matmul.  Layer-0 precision stays ~fp32.
  * Layers 1-2: single bf16 activations (end-to-end rel err ~6e-4 vs 2e-2).
- Activations live in [128, 512] chunk tiles (node-within-tile on partitions,
  4 node-tiles side by side in columns), so inter-layer exchange is ONE
  [128, 512] bf16 AllGather payload per chunk and ONE (+1 peer) SBUF load.
- Engine-queue discipline: sync = bulk input DMA + epilogue stores;
  gpsimd = consts + collective triggers ONLY (so AllGathers pipeline);
  vector = AG-dependent SBUF loads (the only engine that may block on AGs).
- A dummy AllGather issued first absorbs the one-time CC bootstrap (~13 us).
"""

import sys

if "/opt/trn_rl_repo" not in sys.path:  # PYTHONPATH normally provides it
    sys.path.insert(0, "/opt/trn_rl_repo")

import contextlib
import ctypes
import types

import numpy as np
import ml_dtypes

import concourse.bass as bass
import concourse.tile as tile
from concourse import mybir
from concourse.vector_clock import ScopedClock
import concourse.bass_utils as bass_utils
from concourse.bass_utils import run_bass_kernel_spmd

# ---------------------------------------------------------------------------
# Workaround: the walrus build in this container rejects instructions with
# more than one sem wait ("Too many sync wait commands").  Tile's final drain
# attaches one wait per live semaphore; split them across chained SP drains.
_MAX_WAITS_PER_INST = 1


def _patched_drain_and_barrier(self, tick_clock, wait_clock):
    nc = self.nc
    drain_inst = nc.sync.drain()
    wait_clock.add_sem_waits(drain_inst.ins, ScopedClock({None: tick_clock.global_clock}))
    si = drain_inst.ins.sync_info
    waits = list(si.on_wait or [])
    if len(waits) > _MAX_WAITS_PER_INST:
        si.on_wait = waits[:_MAX_WAITS_PER_INST]
        rest = waits[_MAX_WAITS_PER_INST:]
        for i in range(0, len(rest), _MAX_WAITS_PER_INST):
            extra = nc.sync.drain()
            extra.ins.sync_info = mybir.SyncInfo(
                on_wait=rest[i : i + _MAX_WAITS_PER_INST], on_update=[]
            )
    nc.all_engine_barrier()
    assert self.sems is not None
    popped = nc._tile_sem_poison_stack.pop()
    assert popped is self._sem_poison
    nc.clear_and_free_semaphores(list(self.sems.allocated().values()))
    nc.all_engine_barrier()


tile.TileContext._drain_and_barrier = _patched_drain_and_barrier


def _legalize_sync_waits(nc, max_waits=_MAX_WAITS_PER_INST):
    """Split instructions carrying more than ``max_waits`` sem waits.

    Engine sequencers process their instruction stream in order and execute
    sem waits before dispatch, so hoisting excess waits onto NoOps placed
    just before the instruction (same engine) is semantics-preserving.
    """
    n_split = 0
    for fn in nc.m.functions:
        for blk in fn.blocks:
            insts = blk.instructions
            i = 0
            while i < len(insts):
                inst = insts[i]
                si = inst.sync_info
                waits = list(si.on_wait) if si and si.on_wait else []
                if len(waits) > max_waits:
                    extra, keep = waits[:-max_waits], waits[-max_waits:]
                    si.on_wait = keep
                    pos = i
                    for j in range(0, len(extra), max_waits):
                        nop = mybir.InstNoOp(name=f"I-lsw{n_split}-{j}", ins=[], outs=[])
                        nop.engine = inst.engine
                        nop.sync_info = mybir.SyncInfo(
                            on_wait=extra[j : j + max_waits], on_update=[]
                        )
                        insts.insert(pos, nop)
                        pos += 1
                        i += 1
                    n_split += 1
                i += 1
    return n_split


# ---------------------------------------------------------------------------
# NTFF profiling hook (antenv.axon_hooks is absent in this image).  Only used
# when run() is called with trace=True; registering it is harmless otherwise.
def _ntff_profile_via_ctypes(so_path):
    try:
        lib = ctypes.CDLL(so_path)
    except OSError:
        return None
    if not hasattr(lib, "axon_start_nrt_profile"):
        return None
    lib.axon_start_nrt_profile.argtypes = [ctypes.POINTER(ctypes.c_int64), ctypes.c_size_t]
    lib.axon_start_nrt_profile.restype = ctypes.c_int64
    lib.axon_stop_nrt_profile.argtypes = [ctypes.c_char_p]
    lib.axon_stop_nrt_profile.restype = ctypes.c_int64

    @contextlib.contextmanager
    def _hook(output_dir, device_ids):
        import jax

        jax.devices()
        if device_ids:
            ids = (ctypes.c_int64 * len(device_ids))(*device_ids)
            rc = lib.axon_start_nrt_profile(ids, len(device_ids))
        else:
            rc = lib.axon_start_nrt_profile(None, 0)
        if rc != 0:
            raise RuntimeError(f"axon_start_nrt_profile rc={rc}")
        try:
            yield
        finally:
            n = lib.axon_stop_nrt_profile(str(output_dir).encode())
            print(f"ntff profile: {n} file(s) written to {output_dir}", file=sys.stderr)

    return _hook


if "antenv.axon_hooks" not in sys.modules:
    _hooks_mod = types.ModuleType("antenv.axon_hooks")
    _hook_inst = _ntff_profile_via_ctypes("/opt/axon/libaxon_pjrt.so")
    _hooks_mod.get_axon_ntff_profile_hook = lambda: _hook_inst
    sys.modules["antenv.axon_hooks"] = _hooks_mod
bass_utils.upload_artifacts = lambda tmpdir: f"local:{tmpdir}"

# ---------------------------------------------------------------------------
B, N, K, C_IN, H, C_OUT = 4, 4096, 2, 32, 64, 32
BN_EPS = 1e-5
N_CORES = 8
HALF = N // 2          # 2048 output nodes per core
NT = N // 128          # 32 node tiles (contraction side)
KC_OUT = [K * H, K * H, K * C_OUT]   # flat output channels per layer: 128,128,64

BF16 = ml_dtypes.bfloat16
FP8 = ml_dtypes.float8_e4m3  # == mybir float8e4 (TRN FP8_EXP4); 0/1 exact

ADJ_FP8 = True   # adjacency in fp8e4 (half the DMA/SBUF); bf16 fallback

_PROGRAM_CACHE = {}


def _build_program(n_layers=3, use_cc=True):
    """Build the SPMD Bass/Tile program (identical on all 8 cores)."""
    nc = bass.Bass("TRN2", target_bir_lowering=False, debug=False, num_devices=N_CORES)
    dt = mybir.dt
    adj_dt = dt.float8e4 if ADJ_FP8 else dt.bfloat16

    # chunk-major adjacency: row kc*128+p, col i*512+j = (adj+I)[node r0+kc*512+j,
    # node i*128+p].  Chunk kc's slab is one contiguous [128, 16384] block.
    adjc_d = nc.dram_tensor("adjc", [4 * 128, NT * 512], adj_dt, kind="ExternalInput").ap()
    # layer-0 x, packed [xh | xl] bf16, block-permuted: col i*128+c = tile i's
    # packed channel c, partition p = node i*128+p.
    x0_d = nc.dram_tensor("x0p", [128, NT * 128], dt.bfloat16, kind="ExternalInput").ap()
    mask_d = nc.dram_tensor("mask_cols", [128, 16], dt.float32, kind="ExternalInput").ap()
    # Wa/Wb are stored block-diagonally over the K=2 slice structure so each
    # MLP stage is a single full-partition matmul with base_partition 0
    # (partition-offset matmul operands crash at runtime on this stack).
    # wa0 additionally stacks the block twice (hi/lo reduction in the MM).
    wa_d = [
        nc.dram_tensor(f"wa{l}", [128, 2 * H], dt.float32, kind="ExternalInput").ap()
        for l in range(3)
    ]
    wb_d = [
        nc.dram_tensor(f"wb{l}", [2 * H, KC_OUT[l]], dt.float32, kind="ExternalInput").ap()
        for l in range(3)
    ]
    ba_d = [
        nc.dram_tensor(f"ba{l}", [128, 1], dt.float32, kind="ExternalInput").ap()
        for l in range(3)
    ]
    dd_d = [
        nc.dram_tensor(f"d{l}", [128, KC_OUT[l]], dt.float32, kind="ExternalInput").ap()
        for l in range(3)
    ]
    out_d = nc.dram_tensor(
        "out", [HALF, KC_OUT[n_layers - 1]], dt.float32, kind="ExternalOutput"
    ).ap()

    with tile.TileContext(nc) as tc:
        with (
            tc.tile_pool(name="const", bufs=1) as cpool,
            tc.tile_pool(name="xf", bufs=2) as xpool,
            tc.tile_pool(name="work", bufs=3) as wpool,
            tc.tile_pool(name="ps_agg", bufs=1, space="PSUM") as ps_agg,
            tc.tile_pool(name="ps_mlp", bufs=2, space="PSUM") as ps_mlp,
            tc.tile_pool(name="dram", bufs=2, space="DRAM") as dpool,
        ):
            # --- HAM warmup: dummy matmuls prime the PE clock to 8/8 just
            # before the first real layer-0 matmuls ---
            wu_lhs = cpool.tile([128, 128], dt.bfloat16, tag="wu_lhs")
            wu_rhs = cpool.tile([128, 512], dt.bfloat16, tag="wu_rhs")
            nc.gpsimd.memset(wu_lhs[:], 0.0)
            nc.gpsimd.memset(wu_rhs[:], 0.0)
            wu_ps = ps_mlp.tile([128, 512], dt.float32, tag="h1")
            for _ in range(8):
                nc.tensor.matmul(wu_ps[:], wu_lhs[:], wu_rhs[:], start=True, stop=True)

            # --- dummy AllGather first: absorbs the one-time collective
            # bootstrap/rendezvous so the first real AG doesn't pay it ---
            if use_cc:
                dummy_in = dpool.tile([2, 16], dt.float32, tag="cc_warm_in", bufs=1)
                dummy_out = dpool.tile([4, 16], dt.float32, tag="cc_warm_out", bufs=1)
                nc.gpsimd.dma_start(dummy_in[:], mask_d[0:2, :])
                nc.gpsimd.collective_compute(
                    "AllGather",
                    mybir.AluOpType.bypass,
                    replica_groups=[[0, 1], [2, 3], [4, 5], [6, 7]],
                    ins=[dummy_in.opt()],
                    outs=[dummy_out.opt()],
                )

            # --- bulk inputs on the sync queue: x0, then adjacency slabs in
            # consumption order (chunk 0 first, split in halves for earlier
            # first-matmul) ---
            x0_sb = cpool.tile([128, NT * 128], dt.bfloat16, tag="x0")
            nc.sync.dma_start(x0_sb[:], x0_d[:])
            adjc_sb = [
                [
                    cpool.tile([128, 16 * 512], adj_dt, tag=f"adjc{kc}h{h}", name=f"adjc_{kc}_{h}")
                    for h in range(2)
                ]
                for kc in range(4)
            ]
            for kc in range(4):
                for h in range(2):
                    nc.sync.dma_start(
                        adjc_sb[kc][h][:],
                        adjc_d[kc * 128 : (kc + 1) * 128, h * 8192 : (h + 1) * 8192],
                    )

            def adj_rhs(kc, i):
                """rhs [128, 512] for contraction tile i of output chunk kc."""
                h, r = divmod(i, 16)
                return adjc_sb[kc][h][:, r * 512 : (r + 1) * 512]

            # --- constants (gpsimd queue) ---
            mask_sb = cpool.tile([128, 16], dt.float32, tag="mask")
            nc.gpsimd.dma_start(mask_sb[:], mask_d[:])
            wa_sb, wb_sb, ba_sb, dd_sb = [], [], [], []
            for l in range(3):
                wa = cpool.tile([128, 2 * H], dt.float32, tag=f"wa{l}")
                nc.gpsimd.dma_start(wa[:], wa_d[l][:])
                wa_sb.append(wa)
                wb = cpool.tile([2 * H, KC_OUT[l]], dt.float32, tag=f"wb{l}")
                nc.gpsimd.dma_start(wb[:], wb_d[l][:])
                wb_sb.append(wb)
                ba = cpool.tile([128, 1], dt.float32, tag=f"ba{l}")
                nc.gpsimd.dma_start(ba[:], ba_d[l][:])
                ba_sb.append(ba)
                dd = cpool.tile([128, KC_OUT[l]], dt.float32, tag=f"d{l}")
                nc.gpsimd.dma_start(dd[:], dd_d[l][:])
                dd_sb.append(dd)

            # xh[l][kc][r]: [128, 512] bf16 chunk activations feeding layer l
            xh = {l: [[None, None] for _ in range(4)] for l in range(1, n_layers)}
            ag_io = {}  # l -> (ag_in, ag_out) for the boundary after layer l

            # layers >=1 consume the previous boundary's AllGathers in their
            # completion order: chunks 0,1 finish with one-stage skew, then
            # the rushed 3, then 2 (see the stage loop below).
            C_ORDER = [0, 1, 3, 2]

            def emit_agg(l, kc):
                """32 accumulating matmuls for output chunk kc of layer l."""
                agg_ps = ps_agg.tile(
                    [128, 512], dt.float32, tag=f"agg{kc}", name=f"agg_l{l}_{kc}"
                )
                if l == 0:
                    seq = [
                        (i, x0_sb[:, i * 128 : (i + 1) * 128]) for i in range(NT)
                    ]
                else:
                    seq = []
                    for c in C_ORDER:
                        for r in range(2):
                            for t in range(4):
                                i = r * 16 + c * 4 + t
                                seq.append(
                                    (i, xh[l][c][r][:, t * 128 : (t + 1) * 128])
                                )
                for ii, (i, lhsT) in enumerate(seq):
                    nc.tensor.matmul(
                        agg_ps[:],
                        lhsT,
                        adj_rhs(kc, i),
                        start=(ii == 0),
                        stop=(ii == NT - 1),
                    )
                return agg_ps

            def emit_agg_copy(agg_ps):
                # psum -> sbuf drain; emitted AFTER the pending chunk's mlp1
                # so the scalar queue runs [relu(s-1), copy(s)]
                agg_sb = wpool.tile([128, 512], dt.float32, tag="agg_sb")
                nc.scalar.copy(agg_sb[:], agg_ps[:])
                return agg_sb

            def emit_mlp1(l, kc, agg_sb):
                h1_ps = ps_mlp.tile([128, 512], dt.float32, tag="h1")
                nc.tensor.matmul(h1_ps[:], wa_sb[l][:], agg_sb[:], start=True, stop=True)
                h1_sb = wpool.tile([128, 512], dt.float32, tag="h1_sb")
                nc.scalar.activation(
                    h1_sb[:],
                    h1_ps[:],
                    mybir.ActivationFunctionType.Relu,
                    bias=ba_sb[l][:, 0:1],
                )
                return h1_sb

            def emit_mlp2(l, kc, h1_sb):
                kco = KC_OUT[l]
                last = l == n_layers - 1
                if not last:
                    ag_in, ag_out = ag_io[l]
                    xst = wpool.tile([128, 512], dt.bfloat16, tag="xst")
                for t in range(4):
                    nsl = slice(t * 128, (t + 1) * 128)
                    xn_ps = ps_mlp.tile([128, kco], dt.float32, tag="xn", bufs=2)
                    nc.tensor.matmul(
                        xn_ps[:], h1_sb[:, nsl], wb_sb[l][:], start=True, stop=True
                    )
                    # (psum + D) * mask
                    xn_sb = wpool.tile([128, kco], dt.float32, tag="xn_sb")
                    nc.vector.tensor_add(xn_sb[:], xn_ps[:], dd_sb[l][:])
                    mcol = mask_sb[:, kc * 4 + t : kc * 4 + t + 1]
                    if not last:
                        nc.scalar.activation(
                            xst[:, t * 128 : t * 128 + kco],
                            xn_sb[:],
                            mybir.ActivationFunctionType.Copy,
                            scale=mcol,
                        )
                    else:
                        xm_sb = wpool.tile([128, kco], dt.float32, tag="xm_sb")
                        nc.scalar.activation(
                            xm_sb[:],
                            xn_sb[:],
                            mybir.ActivationFunctionType.Copy,
                            scale=mcol,
                        )
                        rows = slice((kc * 4 + t) * 128, (kc * 4 + t + 1) * 128)
                        nc.sync.dma_start(out_d[rows, :], xm_sb[:])

                if not last:
                    nc.sync.dma_start(ag_in[kc][:], xst[:])
                    if use_cc:
                        nc.gpsimd.collective_compute(
                            "AllGather",
                            mybir.AluOpType.bypass,
                            replica_groups=[[0, 1], [2, 3], [4, 5], [6, 7]],
                            ins=[ag_in[kc].opt()],
                            outs=[ag_out[kc].opt()],
                        )
                    else:
                        nc.sync.dma_start(ag_out[kc][0:128, :], ag_in[kc][:, :])
                    # next layer's chunk tiles (both pair halves) on gpsimd,
                    # right after their own trigger.  A blocked xh load only
                    # ever waits for an AG already AHEAD of the next trigger's
                    # op in the serial CC stream, so this costs nothing --
                    # unlike putting them on sync, where they starve the next
                    # boundary's ag_in writes (measured +30 us).
                    for r in range(2):
                        xh[l + 1][kc][r] = xpool.tile(
                            [128, 512], dt.bfloat16, tag=f"xh{kc}_{r}", name=f"xh_l{l}_{kc}_{r}"
                        )
                        nc.gpsimd.dma_start(
                            xh[l + 1][kc][r][:], ag_out[kc][r * 128 : (r + 1) * 128, :]
                        )

            for l in range(n_layers - 1):
                ag_io[l] = (
                    [
                        dpool.tile([128, 512], dt.bfloat16, tag=f"ag_in{c}", name=f"ag_in_l{l}_{c}")
                        for c in range(4)
                    ],
                    [
                        dpool.tile([256, 512], dt.bfloat16, tag=f"ag_out{c}", name=f"ag_out_l{l}_{c}")
                        for c in range(4)
                    ],
                )

            # Software-pipelined stage loop.  Normal stages run one-chunk
            # skewed: PE order [32 aggs(s), mlp1(s-1), mlp2(s-1)] so mlp1's
            # scalar-side dependency (the agg copy) completed during the agg
            # burst; mlp2 pays a short relu-latency stall.  The LAST chunk before a layer
            # boundary is "rushed" (epilogue emitted immediately, ~1 us PE
            # stall) because the next layer's first chunk needs its
            # AllGather; the pending chunk's epilogue follows it.
            def emit_epi(l, kc, agg_sb):
                h1_sb = emit_mlp1(l, kc, agg_sb)
                emit_mlp2(l, kc, h1_sb)

            pend = None  # (l, kc, agg_sb)
            for l in range(n_layers):
                rush_layer = l < n_layers - 1
                for kc in range(4):
                    agg_ps = emit_agg(l, kc)
                    h1_pend = None
                    if pend is not None:
                        h1_pend = emit_mlp1(pend[0], pend[1], pend[2])
                    agg_sb = emit_agg_copy(agg_ps)
                    if rush_layer and kc == 3:
                        # rush: own epilogue first (unblocks the boundary AG),
                        # then the pending chunk's mlp2
                        emit_epi(l, kc, agg_sb)
                        if pend is not None:
                            emit_mlp2(pend[0], pend[1], h1_pend)
                        pend = None
                    else:
                        if pend is not None:
                            emit_mlp2(pend[0], pend[1], h1_pend)
                        pend = (l, kc, agg_sb)
            # drain the pipeline (last layer's final chunk)
            if pend is not None:
                emit_epi(pend[0], pend[1], pend[2])

    n_split = _legalize_sync_waits(nc)
    print(f"kernel: legalized {n_split} multi-wait instructions", file=sys.stderr)
    return nc


def get_program():
    if "nc" not in _PROGRAM_CACHE:
        _PROGRAM_CACHE["nc"] = _build_program()
    return _PROGRAM_CACHE["nc"]


def prepare_in_maps(inputs):
    """Host-side prep: fold BN into weights, transpose+slice adjacency, split x."""
    f32 = np.float32
    x = np.asarray(inputs["x"], f32)
    adj = np.asarray(inputs["adj"], f32)
    mask = np.asarray(inputs["mask"]).astype(bool)

    # folded per-layer constants (shared by all cores)
    const = {}
    for l in range(3):
        Wa = np.asarray(inputs[f"Wa{l}"], f32)
        ba = np.asarray(inputs[f"ba{l}"], f32)
        Wb = np.asarray(inputs[f"Wb{l}"], f32)
        bb = np.asarray(inputs[f"bb{l}"], f32)
        s1 = np.asarray(inputs[f"bng{l}"], f32) / np.sqrt(
            np.asarray(inputs[f"bnv{l}"], f32) + BN_EPS
        )
        c1 = np.asarray(inputs[f"bnb{l}"], f32) - np.asarray(inputs[f"bnm{l}"], f32) * s1
        Wb1 = s1[:, None] * Wb
        bb1 = bb + c1 @ Wb
        if l < 2:
            s2 = np.asarray(inputs[f"og{l}"], f32) / np.sqrt(
                np.asarray(inputs[f"ov{l}"], f32) + BN_EPS
            )
            c2 = np.asarray(inputs[f"ob{l}"], f32) - np.asarray(inputs[f"om{l}"], f32) * s2
            Wb2 = (Wb1 * s2[None, :]).astype(f32)
            d = (bb1 * s2 + c2).astype(f32)
        else:
            Wb2 = Wb1.astype(f32)
            d = bb1.astype(f32)
        dtile = np.broadcast_to(
            np.concatenate([d, d])[None, :], (128, 2 * d.shape[0])
        ).copy()
        ci, co = Wa.shape[0], Wb2.shape[1]
        waBD = np.zeros((2 * ci, 2 * H), f32)
        wbBD = np.zeros((2 * H, 2 * co), f32)
        for k in range(2):
            waBD[k * ci : (k + 1) * ci, k * H : (k + 1) * H] = Wa
            wbBD[k * H : (k + 1) * H, k * co : (k + 1) * co] = Wb2
        if l == 0:
            # layer 0: agg psum rows 0:64 = hi part, 64:128 = lo part; stack
            # the 64-row block-diag Wa twice so the MM reduces hi+lo.
            const["wa0"] = np.vstack([waBD, waBD])
        else:
            const[f"wa{l}"] = waBD
        const[f"wb{l}"] = wbBD
        const[f"ba{l}"] = np.concatenate([ba, ba]).reshape(128, 1).astype(f32)
        const[f"d{l}"] = dtile.astype(f32)

    in_maps = []
    for core in range(N_CORES):
        b, half = divmod(core, 2)
        r0 = half * HALF
        # adjT[i, j] = adj[b][r0+j, i] + I
        adjT = np.ascontiguousarray(adj[b][r0 : r0 + HALF, :].T)
        adjT[np.arange(HALF) + r0, np.arange(HALF)] += 1.0
        adjT = adjT.astype(FP8 if ADJ_FP8 else BF16)
        # chunk-major slabs: adjc[kc*128+p, i*512+j] = adjT[i*128+p, kc*512+j]
        adjc = np.ascontiguousarray(
            adjT.reshape(NT, 128, 4, 512).transpose(2, 1, 0, 3).reshape(4 * 128, NT * 512)
        )
        xb = x[b].reshape(N, K * C_IN)
        xh = xb.astype(BF16)
        xl = (xb - xh.astype(f32)).astype(BF16)
        x0p = np.hstack([xh, xl])  # [4096, 128]
        # block-permuted: x0pp[p, i*128+c] = x0p[i*128+p, c]
        x0pp = np.ascontiguousarray(
            x0p.reshape(NT, 128, 128).transpose(1, 0, 2).reshape(128, NT * 128)
        )
        mhalf = mask[b][r0 : r0 + HALF].astype(f32)
        m = dict(const)
        m["adjc"] = adjc
        m["x0p"] = x0pp
        m["mask_cols"] = np.ascontiguousarray(mhalf.reshape(16, 128).T)
        in_maps.append(m)
    return in_maps


def run(in_maps, trace=False, **kw):
    nc = get_program()
    return run_bass_kernel_spmd(nc, in_maps, list(range(N_CORES)), trace=trace, **kw)


def kernel(**inputs) -> np.ndarray:
    in_maps = prepare_in_maps(inputs)
    res = run(in_maps)
    out = np.zeros((B, N, K, C_OUT), np.float32)
    for core in range(N_CORES):
        b, half = divmod(core, 2)
        r0 = half * HALF
        out[b, r0 : r0 + HALF] = res.results[core]["out"].reshape(HALF, K, C_OUT)
    return out
